# revision 2
# baseline (speedup 1.0000x reference)
"""LIF spike recurrence kernel for Trainium2 (8 NeuronCores, SPMD).

Problem: x [32, 128, 32, 32, 8] f32, recurrence over last (time) dim:
    u_t = TAU * u_{t-1} * (1 - o_{t-1}) + x_t
    o_t = 1[u_t - VTH > 0]
Output: o [32, 128, 32, 32, 8] f32 (0.0 / 1.0 spikes).

v2 strategy (exact, measured-roofline driven):
  - Shard batch dim (32) across 8 cores -> 4 per core; pure elementwise, no
    communication. 16.78 MB in + 4.19 MB out per core ~= 62 us DMA floor.
  - Scaled membrane space: v_t = u_t * 4^t (powers of two => bitwise-exact
    rescaling). Since TAU = 0.25, the recurrence collapses to
        v_t = c_{t-1} + x~_t,   c = v * [v <= VTH*4^t],   x~_t = x_t * 4^t
    i.e. the TAU multiply vanishes; host pre-scales x by 4^t (exact).
  - Host pre-transposes input to plane-major tile blocks [P, T, fi] so every
    device op is contiguous (strided SBUF reads cost 1.64x on DVE, strided
    writes 2x on ACT -- measured on the v1 trace).
  - Work split: DVE tiles run mask (STT is_le*mult) + add (TT) = 2 ops/step;
    Pool (gpsimd) tiles run TS + 2 TT per step on a 28% pixel slice, an
    independent chain (no cross-engine ping-pong). ACT computes all spike
    outputs in ONE pass/plane: Sign(4^-t * v - VTH) -> int8 {-1,0,1}
    (4^-t * v == u exactly), host maps > 0. Output stored as int8 (4x less
    store traffic); host converts to f32.
  - Exactness: all rescalings are powers of two; compares use
    theta_t = fl32(0.3) * 4^t so [v > theta_t] <=> [u > fl32(0.3)]; the
    single rounded add per step matches the reference's fl(TAU*c + x).
"""

import numpy as np

TAU = 0.25
VTH = 0.3
N_CORES = 8
P = 128
T = 8
B_LOC = 4  # batches per core
PIX_PER_CORE = B_LOC * 128 * 32 * 32  # 524288
NPP = PIX_PER_CORE // P  # 4096 pixels per partition
COLS = NPP * T  # 32768 flat columns per partition

# (pixels-per-partition, owner) per tile; owners: 'd' = DVE chain, 'p' = Pool
# chain. Pool gets ~28% of pixels (its 3-op form runs ~2.6 cyc/elem for TT).
TILES = [
    (512, "d"),
    (576, "p"),
    (800, "d"),
    (576, "p"),
    (800, "d"),
    (832, "d"),
]
assert sum(fi for fi, _ in TILES) == NPP
# ACT-queue order for sign+store emission: pool tiles pushed later so their
# (slower) chains don't head-of-line-block DVE tiles' signs/stores.
SIGN_ORDER = [0, 2, 1, 4, 5, 3]

_CACHE = {}

_SCALE = (np.float32(4.0) ** np.arange(T, dtype=np.int32)).astype(np.float32)
_THETA = [float(np.float32(VTH) * np.float32(4.0) ** t) for t in range(T)]
_INVS = [float(np.float32(4.0) ** (-t)) for t in range(T)]


def _build_nc():
    import concourse.tile as tile
    from concourse import bacc, mybir

    Alu = mybir.AluOpType
    AF = mybir.ActivationFunctionType
    f32 = mybir.dt.float32
    i8 = mybir.dt.int8

    nc = bacc.Bacc(
        "TRN2",
        target_bir_lowering=False,
        debug=False,
        enable_asserts=False,
        num_devices=N_CORES,
    )
    x_d = nc.dram_tensor("x", [P, COLS], f32, kind="ExternalInput").ap()
    o_d = nc.dram_tensor("o", [P, COLS], i8, kind="ExternalOutput").ap()

    # ACT activation bias needs a pre-registered const AP.
    cb = nc.alloc_sbuf_tensor("const-f32-negvth", [128, 1], f32)
    nc.gpsimd.memset(cb.ap(), -VTH)
    nc.const_aps.aps[(f32, -VTH)] = cb.ap()
    nc.all_engine_barrier()

    with tile.TileContext(nc) as tc:
        with tc.tile_pool(name="xd", bufs=3) as xdp, tc.tile_pool(
            name="xq", bufs=2
        ) as xqp, tc.tile_pool(name="vd", bufs=4) as vdp, tc.tile_pool(
            name="cd", bufs=2
        ) as cdp, tc.tile_pool(name="vq", bufs=8) as vqp, tc.tile_pool(
            name="mq", bufs=2
        ) as mqp, tc.tile_pool(name="cq", bufs=2) as cqp, tc.tile_pool(
            name="sd", bufs=3
        ) as sdp:
            # ---- Phase 1: all loads, prefetch-ordered on the sync HWDGE ring.
            tiles = []
            col = 0
            for fi, owner in TILES:
                pool = xdp if owner == "d" else xqp
                xt = pool.tile([P, T, fi], f32, tag=f"x{owner}")
                nc.sync.dma_start(
                    xt.rearrange("p t f -> p (t f)"), x_d[:, col : col + T * fi]
                )
                tiles.append({"fi": fi, "owner": owner, "col": col, "xt": xt})
                col += T * fi

            # ---- Phase 2: recurrence chains (DVE tiles on vector, pool tiles
            # on gpsimd -- independent per-engine queues).
            for tl in tiles:
                fi, xt = tl["fi"], tl["xt"]
                v_prev = xt[:, 0, :]
                vs = [v_prev]
                if tl["owner"] == "d":
                    for t in range(1, T):
                        c = cdp.tile([P, fi], f32, tag="cd")
                        nc.vector.scalar_tensor_tensor(
                            c[:], v_prev, _THETA[t - 1], v_prev,
                            op0=Alu.is_le, op1=Alu.mult,
                        )
                        vn = vdp.tile([P, fi], f32, tag="vd")
                        nc.vector.tensor_tensor(
                            vn[:], c[:], xt[:, t, :], op=Alu.add
                        )
                        v_prev = vn[:]
                        vs.append(v_prev)
                else:
                    for t in range(1, T):
                        m = mqp.tile([P, fi], f32, tag="mq")
                        nc.gpsimd.tensor_scalar(
                            m[:], v_prev, _THETA[t - 1], None, op0=Alu.is_le
                        )
                        c = cqp.tile([P, fi], f32, tag="cq")
                        nc.gpsimd.tensor_tensor(c[:], v_prev, m[:], op=Alu.mult)
                        vn = vqp.tile([P, fi], f32, tag="vq")
                        nc.gpsimd.tensor_tensor(
                            vn[:], c[:], xt[:, t, :], op=Alu.add
                        )
                        v_prev = vn[:]
                        vs.append(v_prev)
                tl["vs"] = vs

            # ---- Phase 3: spike outputs. One ACT pass per plane:
            # s = Sign(4^-t * v - VTH) -> int8 in {-1,0,1}; host maps (s > 0).
            for idx in SIGN_ORDER:
                tl = tiles[idx]
                fi = tl["fi"]
                s = sdp.tile([P, T, fi], i8, tag="s")
                for t in range(T):
                    nc.scalar.activation(
                        s[:, t, :], tl["vs"][t], AF.Sign,
                        bias=-VTH, scale=_INVS[t],
                    )
                nc.scalar.dma_start(
                    o_d[:, tl["col"] : tl["col"] + T * fi],
                    s.rearrange("p t f -> p (t f)"),
                )
    nc.compile()
    return nc


def _get_nc():
    if "nc" not in _CACHE:
        _CACHE["nc"] = _build_nc()
    return _CACHE["nc"]


def _shard(x: np.ndarray):
    xs = np.ascontiguousarray(x, dtype=np.float32)
    ins = []
    for i in range(N_CORES):
        xc = xs[i * B_LOC : (i + 1) * B_LOC].reshape(P, NPP, T)
        xv = xc * _SCALE  # exact: power-of-two scale per time plane
        buf = np.empty((P, COLS), np.float32)
        off = 0
        col = 0
        for fi, _ in TILES:
            blk = xv[:, off : off + fi, :]  # [P, fi, T]
            buf[:, col : col + T * fi] = blk.transpose(0, 2, 1).reshape(P, T * fi)
            off += fi
            col += T * fi
        ins.append(buf)
    return ins


def _unshard(outs):
    full = np.empty((N_CORES * B_LOC, 128, 32, 32, T), np.float32)
    for i, o in enumerate(outs):  # o: [P, COLS] int8
        oc = np.empty((P, NPP, T), np.int8)
        off = 0
        col = 0
        for fi, _ in TILES:
            blk = o[:, col : col + T * fi].reshape(P, T, fi)
            oc[:, off : off + fi, :] = blk.transpose(0, 2, 1)
            off += fi
            col += T * fi
        full[i * B_LOC : (i + 1) * B_LOC] = (
            (oc > 0).astype(np.float32).reshape(B_LOC, 128, 32, 32, T)
        )
    return full


def _run(in_maps, **kwargs):
    from concourse.bass_utils import run_bass_kernel_spmd

    nc = _get_nc()
    return run_bass_kernel_spmd(nc, in_maps, core_ids=list(range(N_CORES)), **kwargs)


def kernel(x: np.ndarray) -> np.ndarray:
    in_maps = [{"x": s} for s in _shard(x)]
    res = _run(in_maps)
    return _unshard([res.results[i]["o"] for i in range(N_CORES)])


# revision 4
# speedup vs baseline: 2.2236x; 2.2236x over previous
"""LIF spike recurrence kernel for Trainium2 (8 NeuronCores, SPMD).

Problem: x [32, 128, 32, 32, 8] f32, recurrence over last (time) dim:
    u_t = TAU * u_{t-1} * (1 - o_{t-1}) + x_t
    o_t = 1[u_t - VTH > 0]
Output: o [32, 128, 32, 32, 8] f32 (0.0 / 1.0 spikes).

v2 strategy (exact, measured-roofline driven):
  - Shard batch dim (32) across 8 cores -> 4 per core; pure elementwise, no
    communication. 16.78 MB in + 4.19 MB out per core ~= 62 us DMA floor.
  - Scaled membrane space: v_t = u_t * 4^t (powers of two => bitwise-exact
    rescaling). Since TAU = 0.25, the recurrence collapses to
        v_t = c_{t-1} + x~_t,   c = v * [v <= VTH*4^t],   x~_t = x_t * 4^t
    i.e. the TAU multiply vanishes; host pre-scales x by 4^t (exact).
  - Host pre-transposes input to plane-major tile blocks [P, T, fi] so every
    device op is contiguous (strided SBUF reads cost 1.64x on DVE, strided
    writes 2x on ACT -- measured on the v1 trace).
  - Work split: DVE tiles run mask (STT is_le*mult) + add (TT) = 2 ops/step;
    Pool (gpsimd) tiles run TS + 2 TT per step on a 28% pixel slice, an
    independent chain (no cross-engine ping-pong). ACT computes all spike
    outputs in ONE pass/plane: Sign(4^-t * v - VTH) -> int8 {-1,0,1}
    (4^-t * v == u exactly), host maps > 0. Output stored as int8 (4x less
    store traffic); host converts to f32.
  - Exactness: all rescalings are powers of two; compares use
    theta_t = fl32(0.3) * 4^t so [v > theta_t] <=> [u > fl32(0.3)]; the
    single rounded add per step matches the reference's fl(TAU*c + x).
"""

import numpy as np

TAU = 0.25
VTH = 0.3
N_CORES = 8
P = 128
T = 8
B_LOC = 4  # batches per core
PIX_PER_CORE = B_LOC * 128 * 32 * 32  # 524288
NPP = PIX_PER_CORE // P  # 4096 pixels per partition
COLS = NPP * T  # 32768 flat columns per partition

# (pixels-per-partition, owner) per tile. All tiles run the DVE chain: v2
# measured that ANY concurrent GpSimd work poisons DVE throughput ~3x (shared
# SBUF ports) and pool tensor_scalar itself runs ~19 cyc/elem. Pool stays idle.
TILES = [
    (512, "d"),
    (896, "d"),
    (896, "d"),
    (896, "d"),
    (896, "d"),
]
assert sum(fi for fi, _ in TILES) == NPP
SIGN_ORDER = list(range(len(TILES)))

_CACHE = {}

_SCALE = (np.float32(4.0) ** np.arange(T, dtype=np.int32)).astype(np.float32)
_THETA = [float(np.float32(VTH) * np.float32(4.0) ** t) for t in range(T)]
_INVS = [float(np.float32(4.0) ** (-t)) for t in range(T)]


def _build_nc():
    import concourse.tile as tile
    from concourse import bacc, mybir

    Alu = mybir.AluOpType
    AF = mybir.ActivationFunctionType
    f32 = mybir.dt.float32
    i8 = mybir.dt.int8

    nc = bacc.Bacc(
        "TRN2",
        target_bir_lowering=False,
        debug=False,
        enable_asserts=False,
        num_devices=N_CORES,
    )
    x_d = nc.dram_tensor("x", [P, COLS], f32, kind="ExternalInput").ap()
    o_d = nc.dram_tensor("o", [P, COLS], i8, kind="ExternalOutput").ap()

    # ACT activation bias needs a pre-registered const AP.
    cb = nc.alloc_sbuf_tensor("const-f32-negvth", [128, 1], f32)
    nc.gpsimd.memset(cb.ap(), -VTH)
    nc.const_aps.aps[(f32, -VTH)] = cb.ap()
    nc.all_engine_barrier()

    with tile.TileContext(nc) as tc:
        with tc.tile_pool(name="xd", bufs=3) as xdp, tc.tile_pool(
            name="vd", bufs=6
        ) as vdp, tc.tile_pool(name="sd", bufs=3) as sdp:
            # ---- Phase 1: all loads, prefetch-ordered on the sync HWDGE ring.
            tiles = []
            col = 0
            for fi, owner in TILES:
                xt = xdp.tile([P, T, fi], f32, tag="xd")
                nc.sync.dma_start(
                    xt.rearrange("p t f -> p (t f)"), x_d[:, col : col + T * fi]
                )
                tiles.append({"fi": fi, "owner": owner, "col": col, "xt": xt})
                col += T * fi

            # ---- Phase 2: recurrence chains, DVE only. Per step: one STT
            # (mask) into a fresh plane, then an in-place TT add of x~_t.
            for tl in tiles:
                fi, xt = tl["fi"], tl["xt"]
                v_prev = xt[:, 0, :]
                vs = [v_prev]
                for t in range(1, T):
                    vn = vdp.tile([P, fi], f32, tag="vd")
                    nc.vector.scalar_tensor_tensor(
                        vn[:], v_prev, _THETA[t - 1], v_prev,
                        op0=Alu.is_le, op1=Alu.mult,
                    )
                    nc.vector.tensor_tensor(
                        vn[:], vn[:], xt[:, t, :], op=Alu.add
                    )
                    v_prev = vn[:]
                    vs.append(v_prev)
                tl["vs"] = vs

            # ---- Phase 3: spike outputs. One ACT pass per plane:
            # s = Sign(4^-t * v - VTH) -> int8 in {-1,0,1}; host maps (s > 0).
            for idx in SIGN_ORDER:
                tl = tiles[idx]
                fi = tl["fi"]
                s = sdp.tile([P, T, fi], i8, tag="s")
                for t in range(T):
                    nc.scalar.activation(
                        s[:, t, :], tl["vs"][t], AF.Sign,
                        bias=-VTH, scale=_INVS[t],
                    )
                nc.scalar.dma_start(
                    o_d[:, tl["col"] : tl["col"] + T * fi],
                    s.rearrange("p t f -> p (t f)"),
                )
    nc.compile()
    return nc


def _get_nc():
    if "nc" not in _CACHE:
        _CACHE["nc"] = _build_nc()
    return _CACHE["nc"]


def _shard(x: np.ndarray):
    xs = np.ascontiguousarray(x, dtype=np.float32)
    ins = []
    for i in range(N_CORES):
        xc = xs[i * B_LOC : (i + 1) * B_LOC].reshape(P, NPP, T)
        xv = xc * _SCALE  # exact: power-of-two scale per time plane
        buf = np.empty((P, COLS), np.float32)
        off = 0
        col = 0
        for fi, _ in TILES:
            blk = xv[:, off : off + fi, :]  # [P, fi, T]
            buf[:, col : col + T * fi] = blk.transpose(0, 2, 1).reshape(P, T * fi)
            off += fi
            col += T * fi
        ins.append(buf)
    return ins


def _unshard(outs):
    full = np.empty((N_CORES * B_LOC, 128, 32, 32, T), np.float32)
    for i, o in enumerate(outs):  # o: [P, COLS] int8
        oc = np.empty((P, NPP, T), np.int8)
        off = 0
        col = 0
        for fi, _ in TILES:
            blk = o[:, col : col + T * fi].reshape(P, T, fi)
            oc[:, off : off + fi, :] = blk.transpose(0, 2, 1)
            off += fi
            col += T * fi
        full[i * B_LOC : (i + 1) * B_LOC] = (
            (oc > 0).astype(np.float32).reshape(B_LOC, 128, 32, 32, T)
        )
    return full


def _run(in_maps, **kwargs):
    from concourse.bass_utils import run_bass_kernel_spmd

    nc = _get_nc()
    return run_bass_kernel_spmd(nc, in_maps, core_ids=list(range(N_CORES)), **kwargs)


def kernel(x: np.ndarray) -> np.ndarray:
    in_maps = [{"x": s} for s in _shard(x)]
    res = _run(in_maps)
    return _unshard([res.results[i]["o"] for i in range(N_CORES)])


# revision 5
# speedup vs baseline: 2.7615x; 1.2419x over previous
"""LIF spike recurrence kernel for Trainium2 (8 NeuronCores, SPMD).

Problem: x [32, 128, 32, 32, 8] f32, recurrence over last (time) dim:
    u_t = TAU * u_{t-1} * (1 - o_{t-1}) + x_t
    o_t = 1[u_t - VTH > 0]
Output: o [32, 128, 32, 32, 8] f32 (0.0 / 1.0 spikes).

v4 strategy (measured-roofline driven; all transforms bitwise-exact):
  - Shard batch dim (32) across 8 cores -> 4 per core; no communication.
    16.78 MB in + 4.19 MB (int8) out per core ~= 55-60 us DMA floor.
  - Scaled membrane space: v_t = u_t * 4^t (power-of-two => exact). With
    TAU = 0.25 the recurrence collapses to
        v_t = v_{t-1} * [v_{t-1} <= theta_{t-1}] + x~_t
    (theta_t = fl32(0.3) * 4^t, x~_t = x_t * 4^t, both exact scalings; the
    TAU multiply vanishes; one rounded fp32 add per step, same as the ref).
  - Custom DVE op LIF_STEP_ANT: select(v <= theta, v, 0) + x~ fuses the whole
    step into ONE Vector instruction (3 ALU stages of the 8-slice pipe), so
    the recurrence costs 7 passes instead of 14.
  - Host pre-transposes input to plane-major tile blocks [P, T, fi]: every
    device access is contiguous (strided reads cost 1.64x on DVE, strided
    writes 2x on ACT -- measured).
  - Spikes: ACT computes s_t = Sign(4^-t * v_t - VTH) -> int8 {-1,0,1} in one
    pass/plane (4^-t*v == u exactly); host maps s > 0. Last tile's signs run
    on DVE (tensor_scalar is_gt -> int8 {0,1}) to cut the ACT tail.
  - GpSimd stays idle: any concurrent pool work poisons DVE throughput ~3x
    (shared SBUF ports, measured in v2).
  - Bias const for ACT lives in a tile pool and is memset on the vector
    engine, so no all_engine_barrier: the first input load dispatches at
    ~1.5 us instead of ~9 us.
"""

import numpy as np

TAU = 0.25
VTH = 0.3
N_CORES = 8
P = 128
T = 8
B_LOC = 4  # batches per core
PIX_PER_CORE = B_LOC * 128 * 32 * 32  # 524288
NPP = PIX_PER_CORE // P  # 4096 pixels per partition
COLS = NPP * T  # 32768 flat columns per partition

# pixels-per-partition per tile (all run the fused DVE chain). First tile
# small => short pipeline head; last tiles small => short tail.
TILES = [512, 896, 896, 896, 560, 336]
assert sum(TILES) == NPP

_CACHE = {}

_SCALE = (np.float32(4.0) ** np.arange(T, dtype=np.int32)).astype(np.float32)
_THETA = [float(np.float32(VTH) * np.float32(4.0) ** t) for t in range(T)]
_INVS = [float(np.float32(4.0) ** (-t)) for t in range(T)]


def _lif_op():
    """Register (once) and return the fused custom DVE op:
        out = select(in0 <= s0, in0, 0) + in1
    i.e. one LIF step: v_t = v_{t-1}*[v_{t-1} <= theta] + x~_t."""
    if "lif" in _CACHE:
        return _CACHE["lif"]
    import concourse.dve_ops as dve_ops
    from concourse.dve_ops import DveOp
    from concourse.dve_spec import C0, Spec, Src0, Src1, Zero, lower, select
    from concourse.dve_uop import DveOpSpec

    NAME = "LIF_STEP_ANT"
    existing = [op for op in dve_ops.OPS if op.name == NAME]
    if existing:
        _CACHE["lif"] = existing[0]
        return existing[0]

    spec = Spec(
        body=select(Src0 <= C0, Src0, Zero) + Src1,
        reference=lambda in0, in1, s0, s1, imm2: (
            np.where(in0.astype(np.float32) <= s0, in0.astype(np.float32), 0.0)
            + in1.astype(np.float32)
        ).astype(np.float32),
    )
    shas = {}
    for ver in ("v3", "v4"):
        tmp = DveOpSpec(
            name=NAME, opcode=0, uops=lower(spec, ver=ver), rd1_en=True
        )
        shas[ver] = tmp.sha(ver)
    op = DveOp(NAME, spec, subdim=False, uops_sha=shas)
    dve_ops.OPS.append(op)
    dve_ops.CUSTOM_DVE_SPECS[op.name] = op.spec
    dve_ops._SUB_OPCODE_FOR_NAME[op.name] = (
        dve_ops._CUSTOM_DVE_ROW_BASE + len(dve_ops.OPS) - 1
    )
    _CACHE["lif"] = op
    return op


def _build_nc():
    import concourse.tile as tile
    from concourse import bacc, mybir

    Alu = mybir.AluOpType
    AF = mybir.ActivationFunctionType
    f32 = mybir.dt.float32
    i8 = mybir.dt.int8
    lif = _lif_op()

    nc = bacc.Bacc(
        "TRN2",
        target_bir_lowering=False,
        debug=False,
        enable_asserts=False,
        num_devices=N_CORES,
    )
    x_d = nc.dram_tensor("x", [P, COLS], f32, kind="ExternalInput").ap()
    o_d = nc.dram_tensor("o", [P, COLS], i8, kind="ExternalOutput").ap()

    n_tiles = len(TILES)
    with tile.TileContext(nc) as tc:
        with tc.tile_pool(name="xd", bufs=3) as xdp, tc.tile_pool(
            name="vd", bufs=6
        ) as vdp, tc.tile_pool(name="sd", bufs=3) as sdp, tc.tile_pool(
            name="kb", bufs=1
        ) as kbp:
            # ACT bias const, dependency-tracked via the tile pool: no barrier.
            cb = kbp.tile([P, 1], f32, tag="negvth")
            nc.vector.memset(cb[:], -VTH)

            # ---- Phase 1: all loads, prefetch-ordered on the sync HWDGE ring.
            tiles = []
            col = 0
            for fi in TILES:
                xt = xdp.tile([P, T, fi], f32, tag="xd")
                nc.sync.dma_start(
                    xt.rearrange("p t f -> p (t f)"), x_d[:, col : col + T * fi]
                )
                tiles.append({"fi": fi, "col": col, "xt": xt})
                col += T * fi

            # ---- Phase 2: fused recurrence, one DVE op per step.
            for tl in tiles:
                fi, xt = tl["fi"], tl["xt"]
                v_prev = xt[:, 0, :]
                vs = [v_prev]
                for t in range(1, T):
                    vn = vdp.tile([P, fi], f32, tag="vd")
                    nc.vector._custom_dve(
                        lif,
                        out=vn[:],
                        in0=v_prev,
                        in1=xt[:, t, :],
                        s0=_THETA[t - 1],
                    )
                    v_prev = vn[:]
                    vs.append(v_prev)
                tl["vs"] = vs

            # ---- Phase 3: spike outputs + stores. ACT: one Sign pass/plane
            # -> int8 {-1,0,1}. Last tile on DVE (is_gt -> {0,1}) to cut the
            # ACT tail; host maps (s > 0) for both encodings.
            for k, tl in enumerate(tiles):
                fi = tl["fi"]
                s = sdp.tile([P, T, fi], i8, tag="s")
                if k == n_tiles - 1:
                    for t in range(T):
                        nc.vector.tensor_scalar(
                            s[:, t, :], tl["vs"][t], _THETA[t], None,
                            op0=Alu.is_gt,
                        )
                else:
                    for t in range(T):
                        nc.scalar.activation(
                            s[:, t, :], tl["vs"][t], AF.Sign,
                            bias=cb[:, 0:1], scale=_INVS[t],
                        )
                nc.scalar.dma_start(
                    o_d[:, tl["col"] : tl["col"] + T * fi],
                    s.rearrange("p t f -> p (t f)"),
                )
    nc.compile()
    return nc


def _get_nc():
    if "nc" not in _CACHE:
        _CACHE["nc"] = _build_nc()
    return _CACHE["nc"]


def _shard(x: np.ndarray):
    xs = np.ascontiguousarray(x, dtype=np.float32)
    ins = []
    for i in range(N_CORES):
        xc = xs[i * B_LOC : (i + 1) * B_LOC].reshape(P, NPP, T)
        xv = xc * _SCALE  # exact: power-of-two scale per time plane
        buf = np.empty((P, COLS), np.float32)
        off = 0
        col = 0
        for fi in TILES:
            blk = xv[:, off : off + fi, :]  # [P, fi, T]
            buf[:, col : col + T * fi] = blk.transpose(0, 2, 1).reshape(P, T * fi)
            off += fi
            col += T * fi
        ins.append(buf)
    return ins


def _unshard(outs):
    full = np.empty((N_CORES * B_LOC, 128, 32, 32, T), np.float32)
    for i, o in enumerate(outs):  # o: [P, COLS] int8
        oc = np.empty((P, NPP, T), np.int8)
        off = 0
        col = 0
        for fi in TILES:
            blk = o[:, col : col + T * fi].reshape(P, T, fi)
            oc[:, off : off + fi, :] = blk.transpose(0, 2, 1)
            off += fi
            col += T * fi
        full[i * B_LOC : (i + 1) * B_LOC] = (
            (oc > 0).astype(np.float32).reshape(B_LOC, 128, 32, 32, T)
        )
    return full


def _run(in_maps, **kwargs):
    from concourse.bass_utils import run_bass_kernel_spmd

    nc = _get_nc()
    return run_bass_kernel_spmd(nc, in_maps, core_ids=list(range(N_CORES)), **kwargs)


def kernel(x: np.ndarray) -> np.ndarray:
    in_maps = [{"x": s} for s in _shard(x)]
    res = _run(in_maps)
    return _unshard([res.results[i]["o"] for i in range(N_CORES)])


# revision 10
# speedup vs baseline: 2.8014x; 1.0144x over previous
"""LIF spike recurrence kernel for Trainium2 (8 NeuronCores, SPMD).

Problem: x [32, 128, 32, 32, 8] f32, recurrence over last (time) dim:
    u_t = TAU * u_{t-1} * (1 - o_{t-1}) + x_t
    o_t = 1[u_t - VTH > 0]
Output: o [32, 128, 32, 32, 8] f32 (0.0 / 1.0 spikes).

v5 strategy (measured-roofline driven; all transforms bitwise-exact):
  - Shard batch dim (32) across 8 cores -> 4 per core; no communication.
  - Scaled membrane space: v_t = u_t * 4^t (power-of-two => exact). With
    TAU = 0.25 the recurrence collapses to
        v_t = v_{t-1} * [v_{t-1} <= theta_{t-1}] + x~_t
    (theta_t = fl32(0.3)*4^t, x~_t = x_t*4^t; one rounded fp32 add per step,
    identical to the reference's fl(TAU*c + x)).
  - Custom DVE op LIF_STEP_ANT fuses the whole step into ONE Vector
    instruction: 7 passes total for the recurrence.
  - Host pre-transposes input to plane-major tile blocks [P, T, fi]; every
    device access is contiguous.
  - Output BIT-PACKED to 1 byte per pixel (8 timesteps): spike planes
    o_t = [v_t > theta_t] in {0,1} bf16 (DVE tensor_scalar is_gt for 5
    planes, ACT Sign+Relu 2-pass for 3), then the idle TensorEngine computes
    acc = sum_t 2^t * o_t via 8 accumulating matmuls with stationary
    2^t * I (bf16, exact), ACT copies PSUM -> uint8 SBUF, store is 0.52 MB
    instead of 4.19 MB (v4). Host unpacks bits. Everything summed is a small
    integer => exact.
  - GpSimd stays idle (concurrent pool work poisons DVE ~3x, measured).
"""

import contextlib

import numpy as np

TAU = 0.25
VTH = 0.3
N_CORES = 8
P = 128
T = 8
B_LOC = 4  # batches per core
PIX_PER_CORE = B_LOC * 128 * 32 * 32  # 524288
NPP = PIX_PER_CORE // P  # 4096 pixels per partition
COLS = NPP * T  # 32768 flat columns per partition

FI = 512  # pixels per tile per partition == one PSUM bank of f32
N_TILES = NPP // FI  # 8
# spike planes computed by ACT (Sign+Relu 2-pass); rest on DVE (is_gt, 2x).
ACT_PLANES = (1, 2, 3)

_CACHE = {}

_SCALE = (np.float32(4.0) ** np.arange(T, dtype=np.int32)).astype(np.float32)
_THETA = [float(np.float32(VTH) * np.float32(4.0) ** t) for t in range(T)]
_INVS = [float(np.float32(4.0) ** (-t)) for t in range(T)]


def _lif_op():
    """Register (once) and return the fused custom DVE op:
        out = select(in0 <= s0, in0, 0) + in1   (one LIF step)."""
    if "lif" in _CACHE:
        return _CACHE["lif"]
    import concourse.dve_ops as dve_ops
    from concourse.dve_ops import DveOp
    from concourse.dve_spec import C0, Spec, Src0, Src1, Zero, lower, select
    from concourse.dve_uop import DveOpSpec

    NAME = "LIF_STEP_ANT"
    existing = [op for op in dve_ops.OPS if op.name == NAME]
    if existing:
        _CACHE["lif"] = existing[0]
        return existing[0]

    spec = Spec(
        body=select(Src0 <= C0, Src0, Zero) + Src1,
        reference=lambda in0, in1, s0, s1, imm2: (
            np.where(in0.astype(np.float32) <= s0, in0.astype(np.float32), 0.0)
            + in1.astype(np.float32)
        ).astype(np.float32),
    )
    shas = {}
    for ver in ("v3", "v4"):
        tmp = DveOpSpec(
            name=NAME, opcode=0, uops=lower(spec, ver=ver), rd1_en=True
        )
        shas[ver] = tmp.sha(ver)
    op = DveOp(NAME, spec, subdim=False, uops_sha=shas)
    dve_ops.OPS.append(op)
    dve_ops.CUSTOM_DVE_SPECS[op.name] = op.spec
    dve_ops._SUB_OPCODE_FOR_NAME[op.name] = (
        dve_ops._CUSTOM_DVE_ROW_BASE + len(dve_ops.OPS) - 1
    )
    _CACHE["lif"] = op
    return op


def _build_nc():
    import concourse.tile as tile
    from concourse import bacc, mybir

    Alu = mybir.AluOpType
    AF = mybir.ActivationFunctionType
    f32 = mybir.dt.float32
    bf16 = mybir.dt.bfloat16
    u8 = mybir.dt.uint8
    lif = _lif_op()

    nc = bacc.Bacc(
        "TRN2",
        target_bir_lowering=False,
        debug=False,
        enable_asserts=False,
        num_devices=N_CORES,
    )
    x_d = nc.dram_tensor("x", [P, COLS], f32, kind="ExternalInput").ap()
    w_d = nc.dram_tensor("w", [P, T * P], bf16, kind="ExternalInput").ap()
    o_d = nc.dram_tensor("o", [P, NPP], u8, kind="ExternalOutput").ap()

    ctx = contextlib.ExitStack()
    with tile.TileContext(nc) as tc:
        with tc.tile_pool(name="xd", bufs=4) as xdp, tc.tile_pool(
            name="vd", bufs=6
        ) as vdp, tc.tile_pool(name="ob", bufs=3) as obp, tc.tile_pool(
            name="st", bufs=2
        ) as stp, tc.tile_pool(name="su", bufs=2) as sup, tc.tile_pool(
            name="wk", bufs=1
        ) as wkp, tc.psum_pool(name="ps", bufs=2) as psp:
            # ACT bias const, dependency-tracked via the tile pool: no barrier.
            cb = wkp.tile([P, 1], f32, tag="negvth")
            nc.vector.memset(cb[:], -VTH)
            # Stationary pack weights: w[:, t*128:(t+1)*128] = 2^t * I (bf16).
            wt = wkp.tile([P, T * P], bf16, tag="w")
            nc.sync.dma_start(wt[:], w_d[:])

            # ---- Phase 1: all input loads, prefetch-ordered on sync HWDGE.
            tiles = []
            for k in range(N_TILES):
                xt = xdp.tile([P, T, FI], f32, tag="xd")
                nc.sync.dma_start(
                    xt.rearrange("p t f -> p (t f)"),
                    x_d[:, k * T * FI : (k + 1) * T * FI],
                )
                tiles.append({"xt": xt})

            # ---- Phase 2: fused recurrence, one DVE op per step.
            for tl in tiles:
                xt = tl["xt"]
                v_prev = xt[:, 0, :]
                vs = [v_prev]
                for t in range(1, T):
                    vn = vdp.tile([P, FI], f32, tag="vd")
                    nc.vector._custom_dve(
                        lif,
                        out=vn[:],
                        in0=v_prev,
                        in1=xt[:, t, :],
                        s0=_THETA[t - 1],
                    )
                    v_prev = vn[:]
                    vs.append(v_prev)
                tl["vs"] = vs

            # ---- Phase 3: spike planes {0,1} bf16, split DVE / ACT.
            for tl in tiles:
                ob = obp.tile([P, T, FI], bf16, tag="ob")
                for t in range(T):
                    if t in ACT_PLANES:
                        st_ = stp.tile([P, FI], bf16, tag="st")
                        nc.scalar.activation(
                            st_[:], tl["vs"][t], AF.Sign,
                            bias=cb[:, 0:1], scale=_INVS[t],
                        )
                        nc.scalar.activation(ob[:, t, :], st_[:], AF.Relu)
                    else:
                        nc.vector.tensor_scalar(
                            ob[:, t, :], tl["vs"][t], _THETA[t], None,
                            op0=Alu.is_gt,
                        )
                tl["ob"] = ob

            # ---- Phase 4: PE pack (acc = sum_t 2^t * o_t in PSUM), copy to
            # uint8 on ACT, store on the scalar HWDGE ring.
            for k, tl in enumerate(tiles):
                ob = tl["ob"]
                ps = psp.tile([P, FI], f32, tag="ps")
                for j, t in enumerate(range(T)):
                    nc.tensor.matmul(
                        ps[:],
                        wt[:, t * P : (t + 1) * P],
                        ob[:, t, :],
                        start=(j == 0),
                        stop=(j == T - 1),
                    )
                su = sup.tile([P, FI], u8, tag="su")
                nc.scalar.activation(su[:], ps[:], AF.Copy)
                nc.scalar.dma_start(o_d[:, k * FI : (k + 1) * FI], su[:])
    nc.compile()
    ctx.close()
    return nc


def _get_nc():
    if "nc" not in _CACHE:
        _CACHE["nc"] = _build_nc()
    return _CACHE["nc"]


def _pack_weights():
    from concourse import mybir

    w = np.zeros((P, T * P), np.float32)
    for t in range(T):
        w[:, t * P : (t + 1) * P] = np.eye(P, dtype=np.float32) * (2.0**t)
    # powers of two <= 128 are exact in bf16
    return w.astype(mybir.dt.np(mybir.dt.bfloat16))


def _shard(x: np.ndarray):
    xs = np.ascontiguousarray(x, dtype=np.float32)
    wq = _pack_weights()
    ins = []
    for i in range(N_CORES):
        xc = xs[i * B_LOC : (i + 1) * B_LOC].reshape(P, NPP, T)
        xv = xc * _SCALE  # exact: power-of-two scale per time plane
        buf = np.empty((P, COLS), np.float32)
        for k in range(N_TILES):
            blk = xv[:, k * FI : (k + 1) * FI, :]  # [P, FI, T]
            buf[:, k * T * FI : (k + 1) * T * FI] = blk.transpose(
                0, 2, 1
            ).reshape(P, T * FI)
        ins.append({"x": buf, "w": wq})
    return ins


def _unshard(outs):
    full = np.empty((N_CORES * B_LOC, 128, 32, 32, T), np.float32)
    for i, o in enumerate(outs):  # o: [P, NPP] uint8, bit t = spike at t
        bits = np.unpackbits(
            o.reshape(P, NPP, 1), axis=2, bitorder="little"
        )[:, :, :T]
        full[i * B_LOC : (i + 1) * B_LOC] = bits.astype(np.float32).reshape(
            B_LOC, 128, 32, 32, T
        )
    return full


def _run(in_maps, **kwargs):
    from concourse.bass_utils import run_bass_kernel_spmd

    nc = _get_nc()
    return run_bass_kernel_spmd(nc, in_maps, core_ids=list(range(N_CORES)), **kwargs)


def kernel(x: np.ndarray) -> np.ndarray:
    in_maps = _shard(x)
    res = _run(in_maps)
    return _unshard([res.results[i]["o"] for i in range(N_CORES)])
